# revision 1
# baseline (speedup 1.0000x reference)
"""Multi-head attention (B=2, S=2048, D=1024, H=16) on 8 TRN2 NeuronCores.

Sharding: tensor-parallel on heads (2 heads = 128 channels per core).
Everything on-device runs in "transposed" layout [channel, B*S]:
  - host passes hiddenT [D, B*S] (bf16) replicated to all cores
  - per-core Q/K/V projections produce qT/kT/vT [128, B*S]
  - attention per (batch, head) in scoresT layout [key, query]:
      scoresT tile via matmul contracting the head dim, exp on ScalarE
      with the mask as per-partition bias and 1/sqrt(hd) as scale, the
      softmax denominator via an all-ones row appended to V (row 64 of
      the PV accumulator), normalization by DMA-broadcast reciprocal.
  - normalized ctxT (bf16) is AllGathered across cores in 512-column
    chunks; each core then computes a 128-row slice of outT = Wo @
    ctx.T per chunk and returns it.
Host concatenates the 8 slices and transposes back to [B, S, D].

Phase emission order is chosen so the TensorE always has ready "filler"
matmuls (projections for the other batch, output projection for the
previous batch) during the ScalarE-bound attention inner loop — this
both hides those phases and keeps the PE HAM clock-gate warm.
"""

import numpy as np
import ml_dtypes

import concourse.bass as bass
import concourse.mybir as mybir
import concourse.tile as tile
from concourse import bacc
from concourse import bass_utils
from concourse.masks import make_identity

F32 = mybir.dt.float32
BF16 = mybir.dt.bfloat16
BF16_NP = ml_dtypes.bfloat16

B, S, D, H = 2, 2048, 1024, 16
HD = D // H
BS = B * S            # 4096
P = 128               # partitions / channels per core
NCORES = 8
KT = S // P           # 16 key tiles per batch
NQ = 512              # matmul moving free dim
VA_W = HD + 1         # v_aug columns per key tile (64 v cols + ones col)
GC = 512              # normalize / output-projection sub-chunk (columns)
GCC = 1024            # AllGather chunk (columns)

_CACHE = {}


def _build():
    nc = bacc.Bacc("TRN2", target_bir_lowering=False, debug=False,
                   num_devices=NCORES)

    hT = nc.dram_tensor("hT", [D, BS], BF16, kind="ExternalInput")
    wq = nc.dram_tensor("wq", [D, P], BF16, kind="ExternalInput")
    wk = nc.dram_tensor("wk", [D, P], BF16, kind="ExternalInput")
    wv = nc.dram_tensor("wv", [D, P], BF16, kind="ExternalInput")
    wo = nc.dram_tensor("wo", [P, D], BF16, kind="ExternalInput")
    bq = nc.dram_tensor("bq", [P, 1], F32, kind="ExternalInput")
    bk = nc.dram_tensor("bk", [P, 1], F32, kind="ExternalInput")
    bv = nc.dram_tensor("bv", [P, 1], F32, kind="ExternalInput")
    bo = nc.dram_tensor("bo", [P, 1], F32, kind="ExternalInput")
    maskT = nc.dram_tensor("maskT", [S, B], F32, kind="ExternalInput")
    sel = nc.dram_tensor("sel", [2, P], BF16, kind="ExternalInput")
    outT = nc.dram_tensor("outT", [D, BS], F32, kind="ExternalOutput")

    with tile.TileContext(nc) as tc:
        with (
            tc.tile_pool(name="const", bufs=1) as const,
            tc.tile_pool(name="res", bufs=1) as res,
            tc.tile_pool(name="ht", bufs=32) as ht_pool,
            tc.tile_pool(name="va", bufs=2) as va_pool,
            tc.tile_pool(name="pr", bufs=3) as pr_pool,
            tc.tile_pool(name="bc", bufs=2) as bc_pool,
            tc.tile_pool(name="g", bufs=8) as g_pool,
            tc.tile_pool(name="ot", bufs=3) as ot_pool,
            tc.tile_pool(name="dram", bufs=1, space="DRAM") as dram,
            # PSUM: pj 1x[128,512](1 bank, projections+transposes) +
            #       po 1x[128,512](1, output projection) +
            #       sc 2x[128,1024](4) + ctx 1x[65,1024](2) = 8 banks
            tc.tile_pool(name="pj_ps", bufs=1, space="PSUM") as pj_ps,
            tc.tile_pool(name="po_ps", bufs=1, space="PSUM") as po_ps,
            tc.tile_pool(name="sc_ps", bufs=2, space="PSUM") as sc_ps,
            tc.tile_pool(name="ctx_ps", bufs=1, space="PSUM") as ctx_ps,
        ):
            # ---- constants / weights in SBUF ----
            w_sbs = {}
            for nm, w in (("wk", wk), ("wq", wq), ("wv", wv)):
                t = const.tile([P, D], BF16, name=f"{nm}_sb", tag=f"{nm}_sb")
                nc.sync.dma_start(
                    t[:].rearrange("p (j m) -> p j m", j=D // P),
                    w.ap().rearrange("(j p) m -> p j m", p=P))
                w_sbs[nm] = t
            b_sbs = {}
            for nm, bt in (("bq", bq), ("bk", bk), ("bv", bv), ("bo", bo)):
                t = const.tile([P, 1], F32, name=f"{nm}_sb", tag=f"{nm}_sb")
                nc.sync.dma_start(t[:], bt.ap())
                b_sbs[nm] = t
            mask_sb = const.tile([P, B * KT], F32)
            nc.sync.dma_start(
                mask_sb[:].rearrange("p (b t) -> p b t", b=B),
                maskT.ap().rearrange("(t p) b -> p b t", p=P))
            # identity replicated at base partitions 0 and 64 so transposes
            # of head-1 slices (base partition 64) have a matching-base rhs
            ident = const.tile([P, HD], BF16)
            make_identity(nc, ident[0:HD, :])
            nc.sync.dma_start(ident[HD:P, :], ident[0:HD, :])

            qT = res.tile([P, BS], BF16)
            kT = res.tile([P, BS], BF16)
            vT = res.tile([P, BS], BF16)
            ctxraw = res.tile([P, BS], F32)
            ctxn = res.tile([P, BS], BF16)
            # per-(b,h,chunk) softmax-sum tiles: exact deps so the gather
            # chain fires the moment its own chunk's sums exist
            s_sb = {(b, h, c): res.tile([1, 1024], BF16, name=f"s{b}{h}{c}",
                                        tag=f"s{b}{h}{c}")
                    for b in range(B) for h in range(2) for c in range(2)}
            s2_sb = res.tile([2, BS], BF16)  # sums relocated to rows 0/1

            VA = {}

            def setup_va(b):
                vas = []
                for h in range(2):
                    va = va_pool.tile([P, KT * VA_W], BF16, name=f"va{b}{h}",
                                      tag=f"va{h}")
                    nc.vector.memset(va[:], 1.0)
                    vas.append(va)
                VA[b] = vas

            def proj_va_steps(b, nlo, nhi):
                """Projections + v_aug build for column chunks [nlo,nhi) of
                batch b as a generator of small emission steps (PE filler
                inside attention). k first: attention QKs gate on kT."""
                vas = VA[b]
                boff = b * S
                for n in range(b * 4 + nlo, b * 4 + nhi):
                    nsl = bass.ts(n, NQ)
                    hts = []
                    for k in range(D // P):
                        htt = ht_pool.tile([P, NQ], BF16, name=f"ht{k}",
                                           tag="ht")
                        nc.sync.dma_start(htt[:], hT.ap()[bass.ts(k, P), nsl])
                        hts.append(htt)
                    yield
                    for wn, bn, dest in (("wk", "bk", kT), ("wq", "bq", qT),
                                         ("wv", "bv", vT)):
                        ps = pj_ps.tile([P, NQ], F32, name=f"ps_{wn}",
                                        tag="pj")
                        for k in range(D // P):
                            nc.tensor.matmul(
                                ps[:], w_sbs[wn][:, bass.ts(k, P)], hts[k][:],
                                start=(k == 0), stop=(k == D // P - 1))
                            if k % 4 == 3:
                                yield
                        nc.vector.tensor_scalar_add(
                            dest[:, nsl], ps[:], b_sbs[bn][:])
                        yield
                    # vT for this 512-col chunk is done -> its 4 key tiles
                    # can be transposed into v_aug
                    nlocal = n - b * 4
                    for kt in range(nlocal * 4, nlocal * 4 + 4):
                        for h in range(2):
                            hsl = slice(h * HD, (h + 1) * HD)
                            tp = pj_ps.tile([P, HD], BF16, name="tp",
                                            tag="pj")
                            nc.tensor.transpose(
                                tp[:],
                                vT[hsl, boff + kt * P:boff + (kt + 1) * P],
                                ident[hsl, :])
                            nc.vector.tensor_copy(
                                vas[h][:, kt * VA_W:kt * VA_W + HD], tp[:])
                        yield

            def attn_chunk(b, h, c, filler, quiet_head=0, quiet_tail=0,
                           pops=1):
                """one head's attention for a 1024-wide query chunk; pops one
                filler step per key tile to keep the PE stream dense.
                quiet_head/quiet_tail suppress filler pops for the first/last
                N key tiles so gather_norm's DVE ops schedule promptly."""
                va = VA[b][h]
                boff = b * S
                hsl = slice(h * HD, (h + 1) * HD)
                coff = boff + c * 1024
                ctx = ctx_ps.tile([HD + 1, 1024], F32, name="ctx", tag="ctx")
                for kt in range(KT):
                    if filler is not None and quiet_head <= kt < KT - quiet_tail:
                        for _ in range(pops):
                            next(filler, None)
                    sct = sc_ps.tile([P, 1024], F32, name="sct", tag="sct")
                    for i in range(2):
                        nc.tensor.matmul(
                            sct[:, bass.ts(i, NQ)],
                            kT[hsl, boff + kt * P:boff + (kt + 1) * P],
                            qT[hsl, coff + i * NQ:coff + (i + 1) * NQ],
                            start=True, stop=True)
                    pr = pr_pool.tile([P, 1024], BF16, name="pr", tag="pr")
                    nc.scalar.activation(
                        pr[:], sct[:], mybir.ActivationFunctionType.Exp,
                        bias=mask_sb[:, b * KT + kt:b * KT + kt + 1],
                        scale=0.125)
                    for i in range(2):
                        nc.tensor.matmul(
                            ctx[:, bass.ts(i, NQ)],
                            va[:, kt * VA_W:(kt + 1) * VA_W],
                            pr[:, bass.ts(i, NQ)],
                            start=(kt == 0), stop=(kt == KT - 1))
                # fast evacuation: two plain DVE copies release the ctx PSUM
                # slot; reciprocal happens later off-PSUM
                nc.vector.tensor_copy(
                    ctxraw[hsl, coff:coff + 1024], ctx[0:HD, :])
                nc.vector.tensor_copy(
                    s_sb[(b, h, c)][:], ctx[HD:HD + 1, :])

            def gather_norm(b, clo, chi):
                """normalize ctxT and trigger the AllGather for GC-column
                chunks [clo, chi) of batch b. high_priority so the scheduler
                threads this chain in as soon as its deps resolve — the
                collective stream is long and must start ASAP."""
                with tc.high_priority():
                    _gather_norm(b, clo, chi)

            def _gather_norm(b, clo, chi):
                boff = b * S
                for cc in range(clo, chi):
                    for sub in range(GCC // GC):
                        cg = cc * (GCC // GC) + sub
                        goff = boff + cg * GC
                        for h in range(2):
                            nc.gpsimd.dma_start(
                                s2_sb[h:h + 1, goff:goff + GC],
                                s_sb[(b, h, cg // 2)][:, (cg % 2) * GC:
                                                      (cg % 2) * GC + GC])
                        pbc = po_ps.tile([P, GC], F32, name="pbc", tag="po")
                        nc.tensor.matmul(pbc[:], sel_sb[:],
                                         s2_sb[:, goff:goff + GC],
                                         start=True, stop=True)
                        bcr = bc_pool.tile([P, GC], F32, name="bcr",
                                           tag="bcr")
                        nc.vector.reciprocal_approx_fast(bcr[:], pbc[:])
                        nc.vector.tensor_mul(
                            ctxn[:, goff:goff + GC],
                            ctxraw[:, goff:goff + GC], bcr[:])


            CC_OUT = {}

            def oproj_steps(b, clo=0, chi=S // GC, evac_act=False):
                """partial output projection for batch b: outT[o, n] +=
                Wo[o, own chans] @ ctxn — full o range, own 128 channels;
                the cross-core reduction happens on the host (bo too)"""
                boff = b * S
                for cg in range(clo, chi):
                    goff = boff + cg * GC
                    for t in range(D // P):
                        pool = po_ps if t % 2 == 0 else pj_ps
                        po = pool.tile([P, GC], F32, name="po",
                                       tag="pj" if t % 2 else "po")
                        nc.tensor.matmul(
                            po[:], wo_sb[:, bass.ts(t, P)],
                            ctxn[:, goff:goff + GC],
                            start=True, stop=True)
                        ot = ot_pool.tile([P, GC], F32, name="ot", tag="ot")
                        if evac_act and t % 2 == 1:
                            nc.scalar.activation(
                                ot[:], po[:],
                                mybir.ActivationFunctionType.Copy, bias=0.0)
                        else:
                            nc.vector.tensor_copy(ot[:], po[:])
                        nc.sync.dma_start(
                            outT.ap()[bass.ts(t, P), goff:goff + GC], ot[:])
                        if t % 2 == 1:
                            yield
                    yield

            def drain(g):
                for _ in g:
                    pass

            # software pipeline: engines run their streams in-order, so
            # anything that waits on a slow dependency must sit at a stream
            # position where that dependency is already resolved.
            # A(b0) first half up front; the second half is filler inside
            # the very first attention chunk (PE streams are in-order, so
            # without this the first exp waits for every proj matmul)
            setup_va(0)
            drain(proj_va_steps(0, 0, 2))
            # deferred constant loads: not needed until gather_norm/o-proj,
            # so keep them out of the startup DMA burst
            wo_sb = const.tile([P, D], BF16)
            nc.sync.dma_start(wo_sb[:], wo.ap())
            sel_sb = const.tile([2, P], BF16)
            nc.sync.dma_start(sel_sb[:], sel.ap())
            fA0 = proj_va_steps(0, 2, 4)
            attn_chunk(0, 0, 0, fA0, pops=3)
            drain(fA0)
            setup_va(1)
            fillerA = proj_va_steps(1, 0, 4)    # A(b1): filler inside B(b0)
            attn_chunk(0, 1, 0, fillerA, quiet_tail=4)
            gather_norm(0, 0, 1)
            attn_chunk(0, 0, 1, fillerA, quiet_head=4, pops=2)
            attn_chunk(0, 1, 1, fillerA, quiet_tail=4, pops=2)
            gather_norm(0, 1, 2)
            drain(fillerA)                      # b1 inputs must exist pre-B(1)
            fillerB = oproj_steps(0)            # C(b0): filler inside B(b1)
            attn_chunk(1, 0, 0, fillerB, quiet_head=4)
            attn_chunk(1, 1, 0, fillerB, quiet_tail=4)
            gather_norm(1, 0, 1)
            fillerC = oproj_steps(1, 0, 2)      # C(b1) first half as filler
            attn_chunk(1, 0, 1, fillerB, quiet_head=4)
            attn_chunk(1, 1, 1, fillerC, quiet_tail=4)
            gather_norm(1, 1, 2)
            drain(fillerB)
            drain(fillerC)
            drain(oproj_steps(1, 2, 4, evac_act=True))  # C(b1) tail

    nc.compile()
    return nc


def _prep_inputs(hidden_state, attention_mask, Wq, bq, Wk, bk, Wv, bv, Wo, bo):
    h2 = np.ascontiguousarray(
        np.asarray(hidden_state, dtype=np.float32).reshape(BS, D).T
    ).astype(BF16_NP)
    maskT = np.ascontiguousarray(
        np.asarray(attention_mask, dtype=np.float32).reshape(B, S).T)
    selm = np.zeros((2, P), dtype=BF16_NP)
    selm[0, 0:HD] = 1
    selm[1, HD:P] = 1
    in_maps = []
    for c in range(NCORES):
        sl = slice(c * P, (c + 1) * P)
        in_maps.append({
            "hT": h2,
            "wq": np.ascontiguousarray(np.asarray(Wq)[sl, :].T).astype(BF16_NP),
            "wk": np.ascontiguousarray(np.asarray(Wk)[sl, :].T).astype(BF16_NP),
            "wv": np.ascontiguousarray(np.asarray(Wv)[sl, :].T).astype(BF16_NP),
            "wo": np.ascontiguousarray(np.asarray(Wo)[:, sl].T).astype(BF16_NP),
            "bq": np.asarray(bq, dtype=np.float32)[sl].reshape(P, 1),
            "bk": np.asarray(bk, dtype=np.float32)[sl].reshape(P, 1),
            "bv": np.asarray(bv, dtype=np.float32)[sl].reshape(P, 1),
            "bo": np.asarray(bo, dtype=np.float32)[sl].reshape(P, 1),
            "maskT": maskT,
            "sel": selm,
        })
    return in_maps


def kernel(**inputs) -> np.ndarray:
    if "nc" not in _CACHE:
        _CACHE["nc"] = _build()
    nc = _CACHE["nc"]
    in_maps = _prep_inputs(**inputs)
    res = bass_utils.run_bass_kernel_spmd(
        nc, in_maps, core_ids=list(range(NCORES)))
    outT = res.results[0]["outT"].copy()     # [D, BS] partial sums
    for c in range(1, NCORES):
        outT += res.results[c]["outT"]
    out = np.ascontiguousarray(outT.T).reshape(B, S, D)
    out += np.asarray(inputs["bo"], dtype=np.float32)
    return out.astype(np.float32)



# revision 4
# speedup vs baseline: 1.0598x; 1.0598x over previous
"""Multi-head attention (B=2, S=2048, D=1024, H=16) on 8 TRN2 NeuronCores.

Sharding: tensor-parallel on heads (2 heads = 128 channels per core).
Everything on-device runs in "transposed" layout [channel, B*S]:
  - host passes hT pre-arranged so each tensor loads with ONE contiguous
    DMA (hT in 8 column blocks); hT stays resident in SBUF (64KB/part)
  - per-core Q/K/V projections produce qT/kT/vT [128, B*S]
  - attention per (batch, 512-query-block) with BOTH heads packed into
    one [128, 1024] score PSUM tile (h0 cols 0-511, h1 cols 512-1023):
      the two QK matmuls contract hd=64 on disjoint PE row groups
      (partitions 0-63 / 64-127) so they execute CONCURRENTLY (2x);
      ONE exp covers both heads (mask bias is per key-partition, shared);
      per-head PV accumulates into [65, 512] PSUM (ones-row = denom)
  - normalization via DMA-relocated sums + sel matmul broadcast +
    reciprocal, as 512-column blocks right after each query block
  - output projection: partial sums outT[o, n] += Wo[o, own 128 chans]
    @ ctxn, written bf16; host reduces across cores and adds bo.

Phase emission order keeps the TensorE stream dense: projections for
the other batch and output projection for finished blocks are popped as
"filler" inside the ScalarE-bound attention inner loop.

PSUM budget (8 banks): sc 2x[128,1024] (4) + ctx 2x[65,512] (2) +
pj 1x[128,512] (1) + po 1x[128,512] (1).
"""

import numpy as np
import ml_dtypes

import concourse.bass as bass
import concourse.mybir as mybir
import concourse.tile as tile
from concourse import bacc
from concourse import bass_utils
from concourse.masks import make_identity

F32 = mybir.dt.float32
BF16 = mybir.dt.bfloat16
BF16_NP = ml_dtypes.bfloat16

B, S, D, H = 2, 2048, 1024, 16
HD = D // H
BS = B * S            # 4096
P = 128               # partitions / channels per core
NCORES = 8
KT = S // P           # 16 key tiles per batch
NQ = 512              # matmul moving free dim
VA_W = HD + 1         # v_aug columns per key tile (64 v cols + ones col)
QB = 512              # attention query block (ctx PSUM bank width)
NB = S // QB          # 4 query blocks per batch

_CACHE = {}


def _build():
    nc = bacc.Bacc("TRN2", target_bir_lowering=False, debug=False,
                   num_devices=NCORES)

    # all host-side pre-arranged for single contiguous DMAs
    hT = nc.dram_tensor("hT", [P, (D // P) * BS], BF16, kind="ExternalInput")
    wq = nc.dram_tensor("wq", [P, D], BF16, kind="ExternalInput")
    wk = nc.dram_tensor("wk", [P, D], BF16, kind="ExternalInput")
    wv = nc.dram_tensor("wv", [P, D], BF16, kind="ExternalInput")
    wo = nc.dram_tensor("wo", [P, D], BF16, kind="ExternalInput")
    bq = nc.dram_tensor("bq", [P, 1], F32, kind="ExternalInput")
    bk = nc.dram_tensor("bk", [P, 1], F32, kind="ExternalInput")
    bv = nc.dram_tensor("bv", [P, 1], F32, kind="ExternalInput")
    maskP = nc.dram_tensor("maskP", [P, B * KT], F32, kind="ExternalInput")
    sel = nc.dram_tensor("sel", [2, P], BF16, kind="ExternalInput")
    outT = nc.dram_tensor("outT", [D, BS], BF16, kind="ExternalOutput")

    with tile.TileContext(nc) as tc:
        with (
            tc.tile_pool(name="const", bufs=1) as const,
            tc.tile_pool(name="res", bufs=1) as res,
            tc.tile_pool(name="va", bufs=2) as va_pool,
            tc.tile_pool(name="pr", bufs=3) as pr_pool,
            tc.tile_pool(name="bc", bufs=2) as bc_pool,
            tc.tile_pool(name="ot", bufs=3) as ot_pool,
            tc.tile_pool(name="pj_ps", bufs=1, space="PSUM") as pj_ps,
            tc.tile_pool(name="po_ps", bufs=1, space="PSUM") as po_ps,
            tc.tile_pool(name="sc_ps", bufs=2, space="PSUM") as sc_ps,
            tc.tile_pool(name="ctx_ps", bufs=2, space="PSUM") as ctx_ps,
        ):
            # ---- constants / weights in SBUF (contiguous DMAs) ----
            # hT block 0 first: it gates the very first projection matmul
            hT_sb = const.tile([P, (D // P) * BS], BF16)
            BLK = (D // P) * NQ   # 4096 cols per column block
            nc.sync.dma_start(hT_sb[:, 0:BLK], hT.ap()[:, 0:BLK])
            w_sbs = {}
            for nm, w in (("wk", wk), ("wq", wq), ("wv", wv)):
                t = const.tile([P, D], BF16, name=f"{nm}_sb", tag=f"{nm}_sb")
                nc.sync.dma_start(t[:], w.ap())
                w_sbs[nm] = t
            b_sbs = {}
            for nm, bt in (("bq", bq), ("bk", bk), ("bv", bv)):
                t = const.tile([P, 1], F32, name=f"{nm}_sb", tag=f"{nm}_sb")
                nc.sync.dma_start(t[:], bt.ap())
                b_sbs[nm] = t
            mask_sb = const.tile([P, B * KT], F32)
            nc.sync.dma_start(mask_sb[:], maskP.ap())
            for n in range(1, B * NB):
                nc.sync.dma_start(hT_sb[:, n * BLK:(n + 1) * BLK],
                                  hT.ap()[:, n * BLK:(n + 1) * BLK])
            # identity replicated at base partitions 0 and 64 so transposes
            # of head-1 slices (base partition 64) have a matching-base rhs
            ident = const.tile([P, HD], BF16)
            make_identity(nc, ident[0:HD, :])
            nc.sync.dma_start(ident[HD:P, :], ident[0:HD, :])

            qT = res.tile([P, BS], BF16)
            kT = res.tile([P, BS], BF16)
            vT = res.tile([P, BS], BF16)
            ctxraw = res.tile([P, BS], F32)
            ctxn = res.tile([P, BS], BF16)
            # per-(b,h,qblock) softmax-sum tiles: exact deps so the
            # normalize chain fires the moment its own block's sums exist
            s_sb = {(b, h, q): res.tile([1, QB], BF16, name=f"s{b}{h}{q}",
                                        tag=f"s{b}{h}{q}")
                    for b in range(B) for h in range(2) for q in range(NB)}
            s2_sb = res.tile([2, BS], BF16)  # sums relocated to rows 0/1

            VA = {}

            def setup_va(b):
                vas = []
                for h in range(2):
                    va = va_pool.tile([P, KT * VA_W], BF16, name=f"va{b}{h}",
                                      tag=f"va{h}")
                    nc.vector.memset(va[:], 1.0)
                    vas.append(va)
                VA[b] = vas

            def proj_va_steps(b, nlo, nhi):
                """Projections + v_aug build for column chunks [nlo,nhi) of
                batch b as a generator of small emission steps (PE filler
                inside attention). k first: attention QKs gate on kT."""
                vas = VA[b]
                boff = b * S
                for n in range(b * NB + nlo, b * NB + nhi):
                    nsl = bass.ts(n, NQ)
                    for wn, bn, dest in (("wk", "bk", kT), ("wq", "bq", qT),
                                         ("wv", "bv", vT)):
                        ps = pj_ps.tile([P, NQ], F32, name=f"ps_{wn}",
                                        tag="pj")
                        for k in range(D // P):
                            nc.tensor.matmul(
                                ps[:], w_sbs[wn][:, bass.ts(k, P)],
                                hT_sb[:, (n * (D // P) + k) * NQ:
                                      (n * (D // P) + k + 1) * NQ],
                                start=(k == 0), stop=(k == D // P - 1))
                            if k % 4 == 3:
                                yield
                        nc.vector.tensor_scalar_add(
                            dest[:, nsl], ps[:], b_sbs[bn][:])
                        yield
                    # vT for this 512-col chunk is done -> its 4 key tiles
                    # can be transposed into v_aug
                    nlocal = n - b * NB
                    for kt in range(nlocal * 4, nlocal * 4 + 4):
                        for h in range(2):
                            hsl = slice(h * HD, (h + 1) * HD)
                            tp = pj_ps.tile([P, HD], BF16, name="tp",
                                            tag="pj")
                            nc.tensor.transpose(
                                tp[:],
                                vT[hsl, boff + kt * P:boff + (kt + 1) * P],
                                ident[hsl, :])
                            nc.vector.tensor_copy(
                                vas[h][:, kt * VA_W:kt * VA_W + HD], tp[:])
                        yield

            def attn_qb(b, qb, filler, pops=1, quiet_head=0, quiet_tail=0):
                """Both heads' attention for one 512-query block. Each key
                tile: two concurrent row-group QK matmuls into one packed
                [128,1024] score tile, one exp for both heads, two PV
                accumulations. Pops filler steps to keep the PE dense."""
                va0, va1 = VA[b]
                boff = b * S
                qsl = slice(boff + qb * QB, boff + (qb + 1) * QB)
                ctx0 = ctx_ps.tile([VA_W, QB], F32, name="ctx0", tag="ctx")
                ctx1 = ctx_ps.tile([VA_W, QB], F32, name="ctx1", tag="ctx")
                for kt in range(KT):
                    if filler is not None and \
                            quiet_head <= kt < KT - quiet_tail:
                        for _ in range(pops):
                            next(filler, None)
                    ksl = slice(boff + kt * P, boff + (kt + 1) * P)
                    sct = sc_ps.tile([P, 2 * QB], F32, name="sct", tag="sct")
                    nc.tensor.matmul(sct[:, 0:QB], kT[0:HD, ksl],
                                     qT[0:HD, qsl], start=True, stop=True)
                    nc.tensor.matmul(sct[:, QB:2 * QB], kT[HD:P, ksl],
                                     qT[HD:P, qsl], start=True, stop=True)
                    pr = pr_pool.tile([P, 2 * QB], BF16, name="pr", tag="pr")
                    nc.scalar.activation(
                        pr[:], sct[:], mybir.ActivationFunctionType.Exp,
                        bias=mask_sb[:, b * KT + kt:b * KT + kt + 1],
                        scale=0.125)
                    nc.tensor.matmul(
                        ctx0[:], va0[:, kt * VA_W:(kt + 1) * VA_W],
                        pr[:, 0:QB], start=(kt == 0), stop=(kt == KT - 1))
                    nc.tensor.matmul(
                        ctx1[:], va1[:, kt * VA_W:(kt + 1) * VA_W],
                        pr[:, QB:2 * QB], start=(kt == 0),
                        stop=(kt == KT - 1))
                # fast evacuation: plain DVE copies release the ctx PSUM
                # slots; reciprocal happens later off-PSUM. high_priority so
                # the next block's PV reuse isn't blocked on a lazy DVE.
                with tc.high_priority():
                    nc.vector.tensor_copy(ctxraw[0:HD, qsl], ctx0[0:HD, :])
                    nc.vector.tensor_copy(s_sb[(b, 0, qb)][:],
                                          ctx0[HD:HD + 1, :])
                    nc.vector.tensor_copy(ctxraw[HD:P, qsl], ctx1[0:HD, :])
                    nc.vector.tensor_copy(s_sb[(b, 1, qb)][:],
                                          ctx1[HD:HD + 1, :])

            def gather_norm(b, qb):
                """normalize ctxT for one 512-column block: relocate the two
                heads' sums to rows 0/1, broadcast via sel matmul, recip,
                multiply. high_priority so the chain threads in ASAP."""
                with tc.high_priority():
                    goff = b * S + qb * QB
                    for h in range(2):
                        nc.gpsimd.dma_start(s2_sb[h:h + 1, goff:goff + QB],
                                            s_sb[(b, h, qb)][:])
                    pbc = po_ps.tile([P, QB], F32, name="pbc", tag="po")
                    nc.tensor.matmul(pbc[:], sel_sb[:],
                                     s2_sb[:, goff:goff + QB],
                                     start=True, stop=True)
                    bcr = bc_pool.tile([P, QB], F32, name="bcr", tag="bcr")
                    nc.vector.reciprocal_approx_fast(bcr[:], pbc[:])
                    nc.vector.tensor_mul(
                        ctxn[:, goff:goff + QB],
                        ctxraw[:, goff:goff + QB], bcr[:])

            def oproj_steps(b, blo, bhi, evac_act=False):
                """partial output projection for 512-col blocks [blo,bhi) of
                batch b: outT[o, n] += Wo[o, own chans] @ ctxn — full o
                range, own 128 channels; cross-core reduction on host."""
                boff = b * S
                for cg in range(blo, bhi):
                    goff = boff + cg * QB
                    for t in range(D // P):
                        pool = po_ps if t % 2 == 0 else pj_ps
                        po = pool.tile([P, QB], F32, name="po",
                                       tag="pj" if t % 2 else "po")
                        nc.tensor.matmul(
                            po[:], wo_sb[:, bass.ts(t, P)],
                            ctxn[:, goff:goff + QB],
                            start=True, stop=True)
                        ot = ot_pool.tile([P, QB], BF16, name="ot", tag="ot")
                        if evac_act and t % 2 == 1:
                            nc.scalar.activation(
                                ot[:], po[:],
                                mybir.ActivationFunctionType.Copy, bias=0.0)
                        else:
                            nc.vector.tensor_copy(ot[:], po[:])
                        nc.sync.dma_start(
                            outT.ap()[bass.ts(t, P), goff:goff + QB], ot[:])
                        if t % 2 == 1:
                            yield
                    yield

            class FQ:
                """Filler queue: generators become poppable only once
                pushed, so a filler that reads a region (e.g. o-proj on
                ctxn) is never EMITTED before its producer (gather_norm)
                — Tile deps are emission-order-based."""

                def __init__(self):
                    self.gens = []

                def push(self, g):
                    self.gens.append(g)

                def __next__(self):
                    while self.gens:
                        try:
                            return next(self.gens[0])
                        except StopIteration:
                            self.gens.pop(0)
                    return None

            def drain(g):
                if isinstance(g, FQ):
                    while g.gens:
                        next(g)
                    return
                for _ in g:
                    pass

            # software pipeline: engines run their streams in-order, so
            # anything that waits on a slow dependency must sit at a stream
            # position where that dependency is already resolved.
            setup_va(0)
            drain(proj_va_steps(0, 0, 2))
            # deferred constant loads: not needed until gather_norm/o-proj,
            # so keep them out of the startup DMA burst
            wo_sb = const.tile([P, D], BF16)
            nc.sync.dma_start(wo_sb[:], wo.ap())
            sel_sb = const.tile([2, P], BF16)
            nc.sync.dma_start(sel_sb[:], sel.ap())
            # b0 chunks 2-3 projected as filler inside the first attention
            # block (QK(kt) gates on kT chunk kt//4 produced just in time)
            fA0 = proj_va_steps(0, 2, 4)
            attn_qb(0, 0, fA0, pops=3)
            drain(fA0)
            gather_norm(0, 0)
            setup_va(1)
            # b1 proj as filler in A(b0). quiet_head on qb1: b1's hT blocks
            # are still in flight (~24us HBM floor for the 8MB input); a
            # DMA-gated filler matmul would head-of-line-block the QKs.
            fillerA = proj_va_steps(1, 0, 4)
            attn_qb(0, 1, fillerA, pops=2, quiet_head=10)
            gather_norm(0, 1)
            attn_qb(0, 2, fillerA, pops=2)
            gather_norm(0, 2)
            attn_qb(0, 3, fillerA, pops=2, quiet_tail=2)
            gather_norm(0, 3)
            drain(fillerA)                      # b1 inputs must exist
            fq = FQ()                           # oproj: filler in A(b1)
            fq.push(oproj_steps(0, 0, NB))
            attn_qb(1, 0, fq, pops=1, quiet_head=6)
            gather_norm(1, 0)
            fq.push(oproj_steps(1, 0, 1))
            attn_qb(1, 1, fq, pops=1)
            gather_norm(1, 1)
            fq.push(oproj_steps(1, 1, 2))
            attn_qb(1, 2, fq, pops=1)
            gather_norm(1, 2)
            fq.push(oproj_steps(1, 2, 3))
            attn_qb(1, 3, fq, pops=1, quiet_tail=2)
            gather_norm(1, 3)
            drain(fq)
            drain(oproj_steps(1, NB - 1, NB, evac_act=True))  # tail

    nc.compile()
    return nc


def _prep_inputs(hidden_state, attention_mask, Wq, bq, Wk, bk, Wv, bv, Wo,
                 bo):
    # hT blocks: hTr[p, (n*8+k)*512 + m] = h2[k*128+p, n*512+m]
    h2 = np.ascontiguousarray(
        np.asarray(hidden_state, dtype=np.float32).reshape(BS, D).T)
    h3 = h2.reshape(D // P, P, B * NB, NQ)          # [k, p, n, m]
    hTr = np.ascontiguousarray(
        h3.transpose(1, 2, 0, 3).reshape(P, (D // P) * BS)).astype(BF16_NP)
    # mask: maskP[p, b*KT + t] = mask[b, t*128+p]
    m2 = np.asarray(attention_mask, dtype=np.float32).reshape(B, S)
    maskP = np.ascontiguousarray(
        m2.reshape(B, KT, P).transpose(2, 0, 1).reshape(P, B * KT))
    selm = np.zeros((2, P), dtype=BF16_NP)
    selm[0, 0:HD] = 1
    selm[1, HD:P] = 1

    def warr(Wslice):
        # w[p, k*128 + c] = Wslice.T[k*128+p, c]
        wt = np.asarray(Wslice, dtype=np.float32).T     # [D, P]
        return np.ascontiguousarray(
            wt.reshape(D // P, P, P).transpose(1, 0, 2).reshape(P, D)
        ).astype(BF16_NP)

    in_maps = []
    for c in range(NCORES):
        sl = slice(c * P, (c + 1) * P)
        in_maps.append({
            "hT": hTr,
            "wq": warr(np.asarray(Wq)[sl, :]),
            "wk": warr(np.asarray(Wk)[sl, :]),
            "wv": warr(np.asarray(Wv)[sl, :]),
            "wo": np.ascontiguousarray(
                np.asarray(Wo, dtype=np.float32)[:, sl].T).astype(BF16_NP),
            "bq": np.asarray(bq, dtype=np.float32)[sl].reshape(P, 1),
            "bk": np.asarray(bk, dtype=np.float32)[sl].reshape(P, 1),
            "bv": np.asarray(bv, dtype=np.float32)[sl].reshape(P, 1),
            "maskP": maskP,
            "sel": selm,
        })
    return in_maps


def kernel(**inputs) -> np.ndarray:
    if "nc" not in _CACHE:
        _CACHE["nc"] = _build()
    nc = _CACHE["nc"]
    in_maps = _prep_inputs(**inputs)
    res = bass_utils.run_bass_kernel_spmd(
        nc, in_maps, core_ids=list(range(NCORES)))
    outT = res.results[0]["outT"].astype(np.float32)  # [D, BS] partials
    for c in range(1, NCORES):
        outT += res.results[c]["outT"].astype(np.float32)
    out = np.ascontiguousarray(outT.T).reshape(B, S, D)
    out += np.asarray(inputs["bo"], dtype=np.float32)
    return out.astype(np.float32)


# revision 15
# speedup vs baseline: 1.0629x; 1.0030x over previous
"""Multi-head attention (B=2, S=2048, D=1024, H=16) on 8 TRN2 NeuronCores.

Sharding: tensor-parallel on heads (2 heads = 128 channels per core).
Everything on-device runs in "transposed" layout [channel, B*S]:
  - host passes hT pre-arranged so each tensor loads with ONE contiguous
    DMA (hT in 8 column blocks); hT stays resident in SBUF (64KB/part)
  - per-core Q/K/V projections produce qT/kT/vT [128, B*S]
  - attention per (batch, 512-query-block) with BOTH heads packed into
    one [128, 1024] score PSUM tile (h0 cols 0-511, h1 cols 512-1023):
      the two QK matmuls contract hd=64 on disjoint PE row groups
      (partitions 0-63 / 64-127) so they execute CONCURRENTLY (2x);
      ONE exp covers both heads (mask bias is per key-partition, shared);
      per-head PV accumulates into [65, 512] PSUM (ones-row = denom)
  - normalization via DMA-relocated sums + sel matmul broadcast +
    reciprocal, as 512-column blocks right after each query block
  - output projection: partial sums outT[o, n] += Wo[o, own 128 chans]
    @ ctxn, written bf16; host reduces across cores and adds bo.

Phase emission order keeps the TensorE stream dense: projections for
the other batch and output projection for finished blocks are popped as
"filler" inside the ScalarE-bound attention inner loop.

PSUM budget (8 banks): sc 2x[128,1024] (4) + ctx 2x[65,512] (2) +
pj 1x[128,512] (1) + po 1x[128,512] (1).
"""

import numpy as np
import ml_dtypes

import concourse.bass as bass
import concourse.mybir as mybir
import concourse.tile as tile
from concourse import bacc
from concourse import bass_utils
from concourse.masks import make_identity

F32 = mybir.dt.float32
BF16 = mybir.dt.bfloat16
BF16_NP = ml_dtypes.bfloat16

B, S, D, H = 2, 2048, 1024, 16
HD = D // H
BS = B * S            # 4096
P = 128               # partitions / channels per core
NCORES = 8
KT = S // P           # 16 key tiles per batch
NQ = 512              # matmul moving free dim
VA_W = HD + 1         # v_aug columns per key tile (64 v cols + ones col)
QB = 512              # attention query block (ctx PSUM bank width)
NB = S // QB          # 4 query blocks per batch

_CACHE = {}


def _build():
    nc = bacc.Bacc("TRN2", target_bir_lowering=False, debug=False,
                   num_devices=NCORES)

    # all host-side pre-arranged for single contiguous DMAs
    hT = nc.dram_tensor("hT", [P, (D // P) * BS], BF16, kind="ExternalInput")
    wq = nc.dram_tensor("wq", [P, D], BF16, kind="ExternalInput")
    wk = nc.dram_tensor("wk", [P, D], BF16, kind="ExternalInput")
    wv = nc.dram_tensor("wv", [P, D], BF16, kind="ExternalInput")
    wo = nc.dram_tensor("wo", [P, D], BF16, kind="ExternalInput")
    bq = nc.dram_tensor("bq", [P, 1], F32, kind="ExternalInput")
    bk = nc.dram_tensor("bk", [P, 1], F32, kind="ExternalInput")
    bv = nc.dram_tensor("bv", [P, 1], F32, kind="ExternalInput")
    maskP = nc.dram_tensor("maskP", [P, B * KT], F32, kind="ExternalInput")
    sel = nc.dram_tensor("sel", [HD + 1, P], BF16, kind="ExternalInput")
    outT = nc.dram_tensor("outT", [D, BS], BF16, kind="ExternalOutput")

    with tile.TileContext(nc) as tc:
        with (
            tc.tile_pool(name="const", bufs=1) as const,
            tc.tile_pool(name="res", bufs=1) as res,
            tc.tile_pool(name="va", bufs=2) as va_pool,
            tc.tile_pool(name="pr", bufs=3) as pr_pool,
            tc.tile_pool(name="bc", bufs=2) as bc_pool,
            tc.tile_pool(name="ot", bufs=3) as ot_pool,
            tc.tile_pool(name="pj_ps", bufs=1, space="PSUM") as pj_ps,
            tc.tile_pool(name="po_ps", bufs=1, space="PSUM") as po_ps,
            tc.tile_pool(name="sc_ps", bufs=2, space="PSUM") as sc_ps,
            tc.tile_pool(name="ctx_ps", bufs=2, space="PSUM") as ctx_ps,
        ):
            # ---- constants / weights in SBUF (contiguous DMAs) ----
            # The 8.75MB input load is HBM-bound per queue (~165GB/s
            # observed), so spread it over all three DGE queues (SP +
            # Activation HWDGE, GpSimd SWDGE). hT block k is needed at
            # roughly (7 + 3k)us; weights gate the very first matmul so
            # they lead the scalar queue.
            hT_sb = const.tile([P, (D // P) * BS], BF16)
            BLK = (D // P) * NQ   # 4096 cols per column block
            w_sbs = {}
            b_sbs = {}

            def _hblk(eng, n):
                eng.dma_start(hT_sb[:, n * BLK:(n + 1) * BLK],
                              hT.ap()[:, n * BLK:(n + 1) * BLK])

            for nm, w in (("wk", wk), ("wq", wq), ("wv", wv)):
                t = const.tile([P, D], BF16, name=f"{nm}_sb", tag=f"{nm}_sb")
                nc.scalar.dma_start(t[:], w.ap())
                w_sbs[nm] = t
            for nm, bt in (("bq", bq), ("bk", bk), ("bv", bv)):
                t = const.tile([P, 1], F32, name=f"{nm}_sb", tag=f"{nm}_sb")
                nc.scalar.dma_start(t[:], bt.ap())
                b_sbs[nm] = t
            _hblk(nc.sync, 0)
            mask_sb = const.tile([P, B * KT], F32)
            nc.gpsimd.dma_start(mask_sb[:], maskP.ap())
            _hblk(nc.gpsimd, 2)
            _hblk(nc.sync, 3)
            _hblk(nc.scalar, 1)
            _hblk(nc.gpsimd, 5)
            _hblk(nc.sync, 6)
            _hblk(nc.scalar, 4)
            _hblk(nc.scalar, 7)
            # full 128x128 identity: one PE transpose handles both heads
            identF = const.tile([P, P], BF16)
            make_identity(nc, identF[:])

            qT = res.tile([P, BS], BF16)
            kT = res.tile([P, BS], BF16)
            vT = res.tile([P, BS], BF16)
            ctxraw = res.tile([P, BS], F32)
            ctxn = res.tile([P, BS], BF16)
            # softmax sums: the ctx evacuation writes h0's ones-row to
            # partition 0 and h1's to partition 64 (DVE cross-partition
            # copies are only legal at multiple-of-64 offsets), so the sel
            # matmul consumes them with no relocation step. Rows 1-63 are
            # memset once and zeroed by sel's 0 coefficients.
            s2_sb = res.tile([HD + 1, BS], BF16)
            nc.vector.memset(s2_sb[:], 0.0)

            VA = {}

            def setup_va(b):
                vas = []
                for h in range(2):
                    va = va_pool.tile([P, KT * VA_W], BF16, name=f"va{b}{h}",
                                      tag=f"va{h}")
                    nc.vector.memset(va[:], 1.0)
                    vas.append(va)
                VA[b] = vas

            def one_proj(wn, bn, dest, n):
                """one projection for one 512-col chunk, yielding after
                every contraction matmul (~0.4us PE each) so filler pops
                stay inside the attention loop's per-kt PE slack."""
                ps = pj_ps.tile([P, NQ], F32, name=f"ps_{wn}", tag="pj")
                for k in range(D // P):
                    nc.tensor.matmul(
                        ps[:], w_sbs[wn][:, bass.ts(k, P)],
                        hT_sb[:, (n * (D // P) + k) * NQ:
                              (n * (D // P) + k + 1) * NQ],
                        start=(k == 0), stop=(k == D // P - 1))
                    if k % 2 == 1:
                        yield
                nc.vector.tensor_scalar_add(
                    dest[:, bass.ts(n, NQ)], ps[:], b_sbs[bn][:])
                yield

            def proj_va_steps(b, nlo, nhi, with_q=True):
                """K/V projections + v_aug build for column chunks
                [nlo,nhi) of batch b (PE filler inside attention). k first:
                attention QKs gate on kT. One [128,128] PE transpose covers
                BOTH heads' v slices; transposes alternate pj/po banks so
                the DVE evacuation never head-of-line-blocks the next one."""
                vas = VA[b]
                boff = b * S
                for n in range(b * NB + nlo, b * NB + nhi):
                    yield from one_proj("wk", "bk", kT, n)
                    yield from one_proj("wv", "bv", vT, n)
                    if with_q:
                        yield from one_proj("wq", "bq", qT, n)
                    nlocal = n - b * NB
                    for kt in range(nlocal * 4, nlocal * 4 + 4):
                        pool = pj_ps if kt % 2 == 0 else po_ps
                        tp = pool.tile([P, P], BF16, name="tp",
                                       tag="pj" if kt % 2 == 0 else "po")
                        nc.tensor.transpose(
                            tp[:], vT[:, boff + kt * P:boff + (kt + 1) * P],
                            identF[:])
                        nc.vector.tensor_copy(
                            vas[0][:, kt * VA_W:kt * VA_W + HD],
                            tp[:, 0:HD])
                        nc.vector.tensor_copy(
                            vas[1][:, kt * VA_W:kt * VA_W + HD],
                            tp[:, HD:P])
                        yield

            def attn_qb(b, qb, filler, pops=1, quiet_head=0, quiet_tail=0):
                """Both heads' attention for one 512-query block. Each key
                tile: two concurrent row-group QK matmuls into one packed
                [128,1024] score tile, one exp for both heads, two PV
                accumulations. Pops filler steps to keep the PE dense."""
                va0, va1 = VA[b]
                boff = b * S
                qsl = slice(boff + qb * QB, boff + (qb + 1) * QB)
                ctx0 = ctx_ps.tile([VA_W, QB], F32, name="ctx0", tag="ctx")
                ctx1 = ctx_ps.tile([VA_W, QB], F32, name="ctx1", tag="ctx")
                for kt in range(KT):
                    if filler is not None and \
                            quiet_head <= kt < KT - quiet_tail:
                        for _ in range(pops):
                            next(filler, None)
                    ksl = slice(boff + kt * P, boff + (kt + 1) * P)
                    sct = sc_ps.tile([P, 2 * QB], F32, name="sct", tag="sct")
                    nc.tensor.matmul(sct[:, 0:QB], kT[0:HD, ksl],
                                     qT[0:HD, qsl], start=True, stop=True)
                    nc.tensor.matmul(sct[:, QB:2 * QB], kT[HD:P, ksl],
                                     qT[HD:P, qsl], start=True, stop=True)
                    pr = pr_pool.tile([P, 2 * QB], BF16, name="pr", tag="pr")
                    nc.scalar.activation(
                        pr[:], sct[:], mybir.ActivationFunctionType.Exp,
                        bias=mask_sb[:, b * KT + kt:b * KT + kt + 1],
                        scale=0.125)
                    nc.tensor.matmul(
                        ctx0[:], va0[:, kt * VA_W:(kt + 1) * VA_W],
                        pr[:, 0:QB], start=(kt == 0), stop=(kt == KT - 1))
                    nc.tensor.matmul(
                        ctx1[:], va1[:, kt * VA_W:(kt + 1) * VA_W],
                        pr[:, QB:2 * QB], start=(kt == 0),
                        stop=(kt == KT - 1))
                # fast evacuation: plain DVE copies release the ctx PSUM
                # slots; reciprocal happens later off-PSUM. high_priority so
                # the next block's PV reuse isn't blocked on a lazy DVE.
                with tc.high_priority():
                    nc.vector.tensor_copy(ctxraw[0:HD, qsl], ctx0[0:HD, :])
                    nc.vector.tensor_copy(s2_sb[0:1, qsl],
                                          ctx0[HD:HD + 1, :])
                    nc.vector.tensor_copy(ctxraw[HD:P, qsl], ctx1[0:HD, :])
                    nc.vector.tensor_copy(s2_sb[HD:HD + 1, qsl],
                                          ctx1[HD:HD + 1, :])

            def gather_norm(b, qb):
                """normalize ctxT for one 512-column block: broadcast the
                two heads' sums (already at partitions 0/1) via sel matmul,
                recip, multiply. high_priority so the chain threads in."""
                with tc.high_priority():
                    goff = b * S + qb * QB
                    pbc = po_ps.tile([P, QB], F32, name="pbc", tag="po")
                    nc.tensor.matmul(pbc[:], sel_sb[:],
                                     s2_sb[:, goff:goff + QB],
                                     start=True, stop=True)
                    bcr = bc_pool.tile([P, QB], F32, name="bcr", tag="bcr")
                    nc.vector.reciprocal_approx_fast(bcr[:], pbc[:])
                    nc.vector.tensor_mul(
                        ctxn[:, goff:goff + QB],
                        ctxraw[:, goff:goff + QB], bcr[:])

            def oproj_steps(b, blo, bhi, evac_act=False):
                """partial output projection for 512-col blocks [blo,bhi) of
                batch b: outT[o, n] += Wo[o, own chans] @ ctxn — full o
                range, own 128 channels; cross-core reduction on host."""
                boff = b * S
                for cg in range(blo, bhi):
                    goff = boff + cg * QB
                    for t in range(D // P):
                        pool = po_ps if t % 2 == 0 else pj_ps
                        po = pool.tile([P, QB], F32, name="po",
                                       tag="pj" if t % 2 else "po")
                        nc.tensor.matmul(
                            po[:], wo_sb[:, bass.ts(t, P)],
                            ctxn[:, goff:goff + QB],
                            start=True, stop=True)
                        ot = ot_pool.tile([P, QB], BF16, name="ot", tag="ot")
                        if evac_act and t % 2 == 1:
                            nc.scalar.activation(
                                ot[:], po[:],
                                mybir.ActivationFunctionType.Copy, bias=0.0)
                        else:
                            nc.vector.tensor_copy(ot[:], po[:])
                        nc.sync.dma_start(
                            outT.ap()[bass.ts(t, P), goff:goff + QB], ot[:])
                        if t % 2 == 1:
                            yield
                    yield

            class FQ:
                """Filler queue: generators become poppable only once
                pushed, so a filler that reads a region (e.g. o-proj on
                ctxn) is never EMITTED before its producer (gather_norm)
                — Tile deps are emission-order-based."""

                def __init__(self):
                    self.gens = []

                def push(self, g):
                    self.gens.append(g)

                def __next__(self):
                    while self.gens:
                        try:
                            return next(self.gens[0])
                        except StopIteration:
                            self.gens.pop(0)
                    return None

            def drain(g):
                if isinstance(g, FQ):
                    while g.gens:
                        next(g)
                    return
                for _ in g:
                    pass

            # software pipeline: engines run their streams in-order, so
            # anything that waits on a slow dependency must sit at a stream
            # position where that dependency is already resolved.
            setup_va(0)
            drain(proj_va_steps(0, 0, 2))
            # deferred constant loads: not needed until gather_norm/o-proj,
            # so keep them out of the startup DMA burst
            wo_sb = const.tile([P, D], BF16)
            nc.sync.dma_start(wo_sb[:], wo.ap())
            sel_sb = const.tile([HD + 1, P], BF16)
            nc.sync.dma_start(sel_sb[:], sel.ap())
            # b0 chunks 2-3 projected as filler inside the first attention
            # block (QK(kt) gates on kT chunk kt//4 produced just in time)
            fA0 = proj_va_steps(0, 2, 4)
            attn_qb(0, 0, fA0, pops=4)
            drain(fA0)
            gather_norm(0, 0)
            setup_va(1)
            # b1 k/v proj as filler in A(b0); b1's q proj (chunks 1-3) is
            # deferred into A(b1) to balance the two windows' PE load.
            # quiet_head on qb1: b1's hT blocks are still in flight (~24us
            # HBM floor for the 8.75MB input); a DMA-gated filler matmul
            # would head-of-line-block the QKs behind it in the PE stream.
            fq0 = FQ()
            fq0.push(proj_va_steps(1, 0, 1))
            fq0.push(proj_va_steps(1, 1, 4, with_q=False))
            fq0.push(one_proj("wq", "bq", qT, NB))   # b1 chunk 0 q
            attn_qb(0, 1, fq0, pops=2, quiet_head=6)
            gather_norm(0, 1)
            attn_qb(0, 2, fq0, pops=2)
            gather_norm(0, 2)
            attn_qb(0, 3, fq0, pops=2, quiet_tail=2)
            gather_norm(0, 3)
            drain(fq0)                          # b1 k/v/q0 must exist
            fq = FQ()                           # A(b1) fillers
            q1 = one_proj("wq", "bq", qT, NB + 1)
            fq.push(q1)
            fq.push(oproj_steps(0, 0, NB))
            attn_qb(1, 0, fq, pops=2, quiet_head=2)
            drain(q1)                           # qT chunk 1 for attn(1,1)
            gather_norm(1, 0)
            fq.push(oproj_steps(1, 0, 1))
            q2 = one_proj("wq", "bq", qT, NB + 2)
            fq.push(q2)
            attn_qb(1, 1, fq, pops=2)
            drain(q2)
            gather_norm(1, 1)
            fq.push(oproj_steps(1, 1, 2))
            q3 = one_proj("wq", "bq", qT, NB + 3)
            fq.push(q3)
            attn_qb(1, 2, fq, pops=2)
            drain(q3)
            gather_norm(1, 2)
            fq.push(oproj_steps(1, 2, 3))
            attn_qb(1, 3, fq, pops=2, quiet_tail=2)
            gather_norm(1, 3)
            drain(fq)
            drain(oproj_steps(1, NB - 1, NB, evac_act=True))  # tail

    nc.compile()
    return nc


def _prep_inputs(hidden_state, attention_mask, Wq, bq, Wk, bk, Wv, bv, Wo,
                 bo):
    # hT blocks: hTr[p, (n*8+k)*512 + m] = h2[k*128+p, n*512+m]
    h2 = np.ascontiguousarray(
        np.asarray(hidden_state, dtype=np.float32).reshape(BS, D).T)
    h3 = h2.reshape(D // P, P, B * NB, NQ)          # [k, p, n, m]
    hTr = np.ascontiguousarray(
        h3.transpose(1, 2, 0, 3).reshape(P, (D // P) * BS)).astype(BF16_NP)
    # mask: maskP[p, b*KT + t] = mask[b, t*128+p]
    m2 = np.asarray(attention_mask, dtype=np.float32).reshape(B, S)
    maskP = np.ascontiguousarray(
        m2.reshape(B, KT, P).transpose(2, 0, 1).reshape(P, B * KT))
    selm = np.zeros((HD + 1, P), dtype=BF16_NP)
    selm[0, 0:HD] = 1
    selm[HD, HD:P] = 1

    def warr(Wslice):
        # w[p, k*128 + c] = Wslice.T[k*128+p, c]
        wt = np.asarray(Wslice, dtype=np.float32).T     # [D, P]
        return np.ascontiguousarray(
            wt.reshape(D // P, P, P).transpose(1, 0, 2).reshape(P, D)
        ).astype(BF16_NP)

    in_maps = []
    for c in range(NCORES):
        sl = slice(c * P, (c + 1) * P)
        in_maps.append({
            "hT": hTr,
            "wq": warr(np.asarray(Wq)[sl, :]),
            "wk": warr(np.asarray(Wk)[sl, :]),
            "wv": warr(np.asarray(Wv)[sl, :]),
            "wo": np.ascontiguousarray(
                np.asarray(Wo, dtype=np.float32)[:, sl].T).astype(BF16_NP),
            "bq": np.asarray(bq, dtype=np.float32)[sl].reshape(P, 1),
            "bk": np.asarray(bk, dtype=np.float32)[sl].reshape(P, 1),
            "bv": np.asarray(bv, dtype=np.float32)[sl].reshape(P, 1),
            "maskP": maskP,
            "sel": selm,
        })
    return in_maps


def kernel(**inputs) -> np.ndarray:
    if "nc" not in _CACHE:
        _CACHE["nc"] = _build()
    nc = _CACHE["nc"]
    in_maps = _prep_inputs(**inputs)
    res = bass_utils.run_bass_kernel_spmd(
        nc, in_maps, core_ids=list(range(NCORES)))
    outT = res.results[0]["outT"].astype(np.float32)  # [D, BS] partials
    for c in range(1, NCORES):
        outT += res.results[c]["outT"].astype(np.float32)
    out = np.ascontiguousarray(outT.T).reshape(B, S, D)
    out += np.asarray(inputs["bo"], dtype=np.float32)
    return out.astype(np.float32)
